# revision 1
# baseline (speedup 1.0000x reference)
"""CompressedSparseAttention Trainium2 kernel (8 NeuronCores).

Sharding: data-parallel over batch (2) x tensor-parallel over head-pairs (4).
Core c handles batch b = c//4 and heads (2g, 2g+1) with g = c%4.
Each core computes its partial output  attn_out[:, hslice] @ wo[:, hslice].T
([2048, 512]); the host sums the 4 partials per batch (the tensor-parallel
all-reduce done on gather).

Layouts inside a core (SBUF partition dim first):
  xT        [512, 2048]   x[b].T, 4 chunks of [128, 2048], fp32r
  qT/kT     [128, 2048]   rows = 2 heads x 64 dims, bf16 after RoPE
  k_cT      [128, 511]    compressed keys (dims on partitions)
  v_aug     16 x [128, 130]  v chunks transposed to [pos, dim] + ones cols
  vc_aug    4 x [128, 130]   v_c chunks transposed to [w, dim] + ones cols
  scores^T  [keys<=128, q]   PSUM; exp'd on ACT; masks via gpsimd affine_select
  av^T      [65, 512]     PSUM per (head, q-block): rows 0-63 = sum exp*v,
                          row 64 = sum exp (denominator via ones column)
"""

import math

import numpy as np

import concourse.bass as bass
import concourse.mybir as mybir
import concourse.tile as tile
from concourse import bacc
from concourse.bass import ds
from concourse.masks import make_identity

B = 2
L = 2048
D = 512
H = 8
HD = 64
RATIO = 8
STRIDE = 4
WINDOW = 128
THETA = 10000.0
LC = (L - RATIO) // STRIDE + 1  # 511
NCORES = 8
NB = L // 512  # 4 q-blocks of 512
NCH = L // 128  # 16 q-chunks of 128
KD = D // 128  # 4 contraction chunks

F32 = mybir.dt.float32
F32R = mybir.dt.float32r
BF16 = mybir.dt.bfloat16
AF = mybir.ActivationFunctionType
ALU = mybir.AluOpType

_CACHE = {}


def _build_nc():
    nc = bacc.Bacc(
        "TRN2",
        target_bir_lowering=False,
        debug=False,
        num_devices=NCORES,
        name="csa",
    )

    # DRAM I/O (per-core views; float32r is bit-identical to float32)
    xT_d = nc.dram_tensor("xT", [D, L], F32R, kind="ExternalInput")
    wqT_d = nc.dram_tensor("wqT", [D, 128], F32R, kind="ExternalInput")
    wkT_d = nc.dram_tensor("wkT", [D, 128], F32R, kind="ExternalInput")
    wvT_d = nc.dram_tensor("wvT", [D, 128], F32R, kind="ExternalInput")
    wkcT_d = nc.dram_tensor("wkcT", [D, 128], F32R, kind="ExternalInput")
    wvcT_d = nc.dram_tensor("wvcT", [D, 128], F32R, kind="ExternalInput")
    woT_d = nc.dram_tensor("woT", [128, D], F32, kind="ExternalInput")
    cosT_d = nc.dram_tensor("cosT", [128, L], F32, kind="ExternalInput")
    sinST_d = nc.dram_tensor("sinST", [128, L], F32, kind="ExternalInput")
    gateb_d = nc.dram_tensor("gateb", [128, RATIO], F32, kind="ExternalInput")
    sink2_d = nc.dram_tensor("sink2", [1, 2], F32, kind="ExternalInput")
    outp_d = nc.dram_tensor("outp", [L, D], F32, kind="ExternalOutput")

    with tile.TileContext(nc) as tc:
        with tc.tile_pool(name="consts", bufs=1) as cp, \
             tc.tile_pool(name="work", bufs=1) as wp, \
             tc.tile_pool(name="ps", bufs=7, space="PSUM") as pp, \
             tc.tile_pool(name="pss", bufs=1, space="PSUM") as pps:

            # ---------------- init: DMA constants ----------------
            xT = []
            for c in range(KD):
                xt = cp.tile([128, L], F32R, tag=f"xt{c}", name=f"xt{c}")
                nc.sync.dma_start(out=xt, in_=xT_d[ds(128 * c, 128), :])
                xT.append(xt)

            def load_w(dram, tag):
                w = []
                for c in range(KD):
                    t = cp.tile([128, 128], F32R, tag=f"{tag}{c}", name=f"{tag}{c}")
                    nc.sync.dma_start(out=t, in_=dram[ds(128 * c, 128), :])
                    w.append(t)
                return w

            wq = load_w(wqT_d, "wq")
            wk = load_w(wkT_d, "wk")
            wv = load_w(wvT_d, "wv")
            wkc = load_w(wkcT_d, "wkc")
            wvc = load_w(wvcT_d, "wvc")

            woT = cp.tile([128, D], F32, tag="woT")
            nc.sync.dma_start(out=woT, in_=woT_d[:, :])
            woT_bf = cp.tile([128, D], BF16, tag="woT_bf")
            nc.scalar.copy(out=woT_bf, in_=woT)

            cosT = cp.tile([128, L], F32, tag="cosT")
            nc.sync.dma_start(out=cosT, in_=cosT_d[:, :])
            sinST = cp.tile([128, L], F32, tag="sinST")
            nc.sync.dma_start(out=sinST, in_=sinST_d[:, :])
            gateb = cp.tile([128, RATIO], F32, tag="gateb")
            nc.sync.dma_start(out=gateb, in_=gateb_d[:, :])
            sink2 = cp.tile([1, 2], F32, tag="sink2")
            nc.sync.dma_start(out=sink2, in_=sink2_d[:, :])

            # exp(sink) broadcast to all partitions
            exps = cp.tile([1, 2], F32, tag="exps")
            nc.scalar.activation(out=exps, in_=sink2, func=AF.Exp)
            expsb = cp.tile([128, 2], F32, tag="expsb")
            nc.gpsimd.partition_broadcast(expsb, exps)

            # identities for PE transpose
            ident_bf = cp.tile([128, 128], BF16, tag="ident_bf")
            make_identity(nc, ident_bf)
            ident_f = cp.tile([128, 128], F32, tag="ident_f")
            make_identity(nc, ident_f)

            # ---------------- P1: projections + RoPE + pooling ----------------
            qT = cp.tile([128, L], BF16, tag="qT")
            kT = cp.tile([128, L], BF16, tag="kT")
            vT_bf = cp.tile([128, L], BF16, tag="vT_bf")
            y_kc = cp.tile([128, L], F32, tag="y_kc")
            y_vc = cp.tile([128, L], F32, tag="y_vc")

            def project(wlist, qb):
                ps = pp.tile([128, 512], F32, tag="bank", name="proj_ps")
                for c in range(KD):
                    nc.tensor.matmul(
                        ps,
                        wlist[c],
                        xT[c][:, ds(512 * qb, 512)],
                        start=(c == 0),
                        stop=(c == KD - 1),
                    )
                return ps

            def rope_block(ps, outT, qb):
                qraw = wp.tile([128, 512], F32, tag="qraw", bufs=2, name="qraw")
                nc.scalar.copy(out=qraw, in_=ps)
                qsw = wp.tile([128, 512], F32, tag="qsw", bufs=2, name="qsw")
                for a, bb in ((0, 32), (32, 0), (64, 96), (96, 64)):
                    nc.gpsimd.tensor_copy(
                        out=qsw[ds(a, 32), :], in_=qraw[ds(bb, 32), :]
                    )
                m1 = wp.tile([128, 512], F32, tag="m1", bufs=2, name="m1")
                nc.vector.tensor_mul(m1, ps, cosT[:, ds(512 * qb, 512)])
                m2 = wp.tile([128, 512], F32, tag="m2", bufs=2, name="m2")
                nc.vector.tensor_mul(m2, qsw, sinST[:, ds(512 * qb, 512)])
                nc.vector.tensor_add(outT[:, ds(512 * qb, 512)], m1, m2)

            for qb in range(NB):
                ps = project(wq, qb)
                rope_block(ps, qT, qb)
            for qb in range(NB):
                ps = project(wk, qb)
                rope_block(ps, kT, qb)
            for qb in range(NB):
                ps = project(wv, qb)
                nc.scalar.copy(out=vT_bf[:, ds(512 * qb, 512)], in_=ps)
            for qb in range(NB):
                ps = project(wkc, qb)
                nc.scalar.copy(out=y_kc[:, ds(512 * qb, 512)], in_=ps)
            for qb in range(NB):
                ps = project(wvc, qb)
                nc.scalar.copy(out=y_vc[:, ds(512 * qb, 512)], in_=ps)

            # pooling: kc/vc[dim, w] = sum_r gate[r] * y[dim, 4w + r]
            def pool(y, out_bf):
                y4 = y.rearrange("p (w r) -> p r w", r=STRIDE)
                acc = [
                    wp.tile([128, LC], F32, tag="poolA", bufs=1, name="poolA"),
                    wp.tile([128, LC], F32, tag="poolB", bufs=1, name="poolB"),
                ]
                nc.vector.tensor_scalar(
                    out=acc[0],
                    in0=y4[:, 0, 0:LC],
                    scalar1=gateb[:, 0:1],
                    scalar2=None,
                    op0=ALU.mult,
                )
                for r in range(1, RATIO):
                    dst = out_bf if r == RATIO - 1 else acc[r % 2]
                    nc.vector.scalar_tensor_tensor(
                        out=dst,
                        in0=y4[:, r % STRIDE, (r // STRIDE):(r // STRIDE) + LC],
                        scalar=gateb[:, ds(r, 1)],
                        in1=acc[(r - 1) % 2],
                        op0=ALU.mult,
                        op1=ALU.add,
                    )

            k_cT = cp.tile([128, LC], BF16, tag="k_cT")
            v_cT = cp.tile([128, LC], BF16, tag="v_cT")
            pool(y_kc, k_cT)
            pool(y_vc, v_cT)

            # transpose v -> v_aug chunks [pos, dim] (+ones col at 64 and 129)
            v_aug = []
            for ch in range(NCH):
                va = cp.tile([128, 130], BF16, tag=f"v_aug{ch}", name=f"v_aug{ch}")
                nc.gpsimd.memset(va, 1.0)
                tp = pps.tile([128, 128], BF16, tag="small", name="tr_ps")
                nc.tensor.transpose(tp, vT_bf[:, ds(128 * ch, 128)], ident_bf)
                nc.vector.tensor_copy(out=va[:, 0:64], in_=tp[:, 0:64])
                nc.vector.tensor_copy(out=va[:, 65:129], in_=tp[:, 64:128])
                v_aug.append(va)

            vc_aug = []
            for ch in range(4):
                wlen = min(128, LC - 128 * ch)  # 128,128,128,127
                va = cp.tile([128, 130], BF16, tag=f"vc_aug{ch}", name=f"vc_aug{ch}")
                nc.gpsimd.memset(va, 1.0)
                tp = pps.tile([128, 128], BF16, tag="small", name="trc_ps")
                nc.tensor.transpose(
                    tp[0:wlen, :], v_cT[:, ds(128 * ch, wlen)], ident_bf
                )
                nc.vector.tensor_copy(out=va[0:wlen, 0:64], in_=tp[0:wlen, 0:64])
                nc.vector.tensor_copy(out=va[0:wlen, 65:129], in_=tp[0:wlen, 64:128])
                vc_aug.append(va)

            # ---------------- P2: attention ----------------
            rec = [cp.tile([128, NCH], F32, tag=f"rec{h}", name=f"rec{h}") for h in range(2)]
            avT = []  # [128, 512] bf16 per q-block: rows 0-63 h0, 64-127 h1
            for qb in range(NB):
                at = cp.tile([128, 512], BF16, tag=f"avT{qb}", name=f"avT{qb}")
                avT.append(at)

            for qb in range(NB):
                for h in range(2):
                    hs = 64 * h
                    qs = qT[ds(hs, 64), ds(512 * qb, 512)]
                    av = pp.tile([65, 512], F32, tag="bank", name=f"av_{qb}_{h}")
                    first_av = [True]

                    def av_mm(lhsT, rhs, cols, stop=False):
                        nc.tensor.matmul(
                            av[:, cols] if cols is not None else av,
                            lhsT,
                            rhs,
                            start=first_av[0],
                            stop=stop,
                            skip_group_check=True,
                        )
                        first_av[0] = False

                    # --- compressed branch ---
                    for wc in range(qb + 1):
                        wlen = min(128, LC - 128 * wc)
                        sc = pp.tile([128, 512], F32, tag="bank", name="sc_ps")
                        nc.tensor.matmul(
                            sc[0:wlen, :],
                            k_cT[ds(hs, 64), ds(128 * wc, wlen)],
                            qs,
                            start=True,
                            stop=True,
                        )
                        ex = wp.tile([128, 512], BF16, tag="exc", bufs=3, name="exc")
                        nc.scalar.activation(
                            out=ex[0:wlen, :], in_=sc[0:wlen, :], func=AF.Exp,
                            scale=0.125,
                        )
                        if wc >= qb - 1:
                            # causal: keep q_rel >= 4*w_rel + 7 - 512*(qb - wc)
                            nc.gpsimd.affine_select(
                                out=ex[0:wlen, :],
                                in_=ex[0:wlen, :],
                                compare_op=ALU.is_ge,
                                fill=0.0,
                                base=-7 + 512 * (qb - wc),
                                pattern=[[1, 512]],
                                channel_multiplier=-4,
                            )
                        av_mm(
                            vc_aug[wc][0:wlen, ds(65 * h, 65)],
                            ex[0:wlen, :],
                            None,
                        )

                    # --- local window branch ---
                    for sub in range(4):
                        c = 4 * qb + sub
                        qcs = qT[ds(hs, 64), ds(128 * c, 128)]
                        wps = pp.tile([128, 256], F32, tag="bank", name="win_ps")
                        if c > 0:
                            nc.tensor.matmul(
                                wps[:, 0:128],
                                kT[ds(hs, 64), ds(128 * (c - 1), 128)],
                                qcs,
                                start=True,
                                stop=True,
                                skip_group_check=True,
                            )
                        nc.tensor.matmul(
                            wps[:, 128:256],
                            kT[ds(hs, 64), ds(128 * c, 128)],
                            qcs,
                            start=True,
                            stop=True,
                            skip_group_check=True,
                        )
                        exw = wp.tile([128, 256], BF16, tag="exw", bufs=3, name="exw")
                        lo = 0 if c > 0 else 128
                        nc.scalar.activation(
                            out=exw[:, lo:256], in_=wps[:, lo:256], func=AF.Exp,
                            scale=0.125,
                        )
                        if c > 0:
                            # prev chunk: keep k_rel > q_rel
                            nc.gpsimd.affine_select(
                                out=exw[:, 0:128],
                                in_=exw[:, 0:128],
                                compare_op=ALU.is_gt,
                                fill=0.0,
                                base=0,
                                pattern=[[-1, 128]],
                                channel_multiplier=1,
                            )
                        # current chunk: keep q_rel >= k_rel
                        nc.gpsimd.affine_select(
                            out=exw[:, 128:256],
                            in_=exw[:, 128:256],
                            compare_op=ALU.is_ge,
                            fill=0.0,
                            base=0,
                            pattern=[[1, 128]],
                            channel_multiplier=-1,
                        )
                        cols = ds(128 * sub, 128)
                        if c > 0:
                            av_mm(
                                v_aug[c - 1][:, ds(65 * h, 65)], exw[:, 0:128], cols
                            )
                        av_mm(
                            v_aug[c][:, ds(65 * h, 65)], exw[:, 128:256], cols,
                            stop=(sub == 3),
                        )

                    # --- denominator -> reciprocal in [q, 1] layout ---
                    drow = wp.tile([1, 512], F32, tag="drow", bufs=2, name="drow")
                    nc.scalar.copy(out=drow, in_=av[64:65, :])
                    dcol = pps.tile([128, 4], F32, tag="small", name="dcol")
                    for c4 in range(4):
                        nc.tensor.transpose(
                            dcol[:, ds(c4, 1)],
                            drow[:, ds(128 * c4, 128)],
                            ident_f[0:1, 0:1],
                        )
                    dsb = wp.tile([128, 4], F32, tag="dsb", bufs=2, name="dsb")
                    nc.vector.tensor_scalar(
                        out=dsb, in0=dcol, scalar1=expsb[:, ds(h, 1)], scalar2=None,
                        op0=ALU.add,
                    )
                    nc.vector.reciprocal(
                        out=rec[h][:, ds(4 * qb, 4)], in_=dsb
                    )

                    # numerator rows -> SBUF (bf16) for the wo matmul
                    nc.scalar.copy(
                        out=avT[qb][ds(hs, 64), :], in_=av[0:64, :]
                    )

            # ---------------- P3: output projection + normalize ----------------
            for qb in range(NB):
                for sub in range(4):
                    c = 4 * qb + sub
                    wo0 = pp.tile([128, 512], F32, tag="bank", name="wo0")
                    nc.tensor.matmul(
                        wo0, avT[qb][0:64, ds(128 * sub, 128)], woT_bf[0:64, :],
                        start=True, stop=True,
                    )
                    wo1 = pp.tile([128, 512], F32, tag="bank", name="wo1")
                    nc.tensor.matmul(
                        wo1, avT[qb][64:128, ds(128 * sub, 128)], woT_bf[64:128, :],
                        start=True, stop=True,
                    )
                    t0 = wp.tile([128, 512], F32, tag="t0", bufs=2, name="t0")
                    nc.scalar.activation(
                        out=t0, in_=wo0, func=AF.Copy, scale=rec[0][:, ds(c, 1)]
                    )
                    osb = wp.tile([128, 512], F32, tag="osb", bufs=3, name="osb")
                    nc.vector.scalar_tensor_tensor(
                        out=osb,
                        in0=wo1,
                        scalar=rec[1][:, ds(c, 1)],
                        in1=t0,
                        op0=ALU.mult,
                        op1=ALU.add,
                    )
                    nc.sync.dma_start(out=outp_d[ds(128 * c, 128), :], in_=osb)

    nc.compile()
    return nc


def _host_prep(inputs):
    """Build the 8 per-core input maps from full inputs."""
    x = np.asarray(inputs["x"], dtype=np.float32)
    wq = np.asarray(inputs["wq"], dtype=np.float32)
    wk = np.asarray(inputs["wk"], dtype=np.float32)
    wv = np.asarray(inputs["wv"], dtype=np.float32)
    wo = np.asarray(inputs["wo"], dtype=np.float32)
    wk_c = np.asarray(inputs["wk_c"], dtype=np.float32)
    wv_c = np.asarray(inputs["wv_c"], dtype=np.float32)
    gate_logits = np.asarray(inputs["gate_logits"], dtype=np.float32)
    sink_logit = np.asarray(inputs["sink_logit"], dtype=np.float32)

    # rope tables
    half = HD // 2
    inv_freq = 1.0 / (THETA ** (np.arange(half, dtype=np.float32) / half))
    t = np.arange(L, dtype=np.float32)
    f = t[:, None] * inv_freq[None, :]  # [L, 32]
    cos32 = np.cos(f).T.astype(np.float32)  # [32, L]
    sin32 = np.sin(f).T.astype(np.float32)
    cosT = np.tile(cos32, (4, 1))  # rows: i%32
    sinST = np.concatenate([-sin32, sin32, -sin32, sin32], axis=0)

    g = np.exp(gate_logits - gate_logits.max())
    g = (g / g.sum()).astype(np.float32)
    gateb = np.broadcast_to(g[None, :], (128, RATIO)).copy()

    in_maps = []
    for core in range(NCORES):
        b, grp = divmod(core, 4)
        sl = slice(128 * grp, 128 * (grp + 1))
        in_maps.append(
            {
                "xT": np.ascontiguousarray(x[b].T),
                "wqT": np.ascontiguousarray(wq[sl, :].T),
                "wkT": np.ascontiguousarray(wk[sl, :].T),
                "wvT": np.ascontiguousarray(wv[sl, :].T),
                "wkcT": np.ascontiguousarray(wk_c[sl, :].T),
                "wvcT": np.ascontiguousarray(wv_c[sl, :].T),
                "woT": np.ascontiguousarray(wo[:, sl].T),
                "cosT": cosT,
                "sinST": sinST,
                "gateb": gateb,
                "sink2": np.ascontiguousarray(
                    sink_logit[2 * grp : 2 * grp + 2, 0][None, :]
                ),
            }
        )
    return in_maps


def kernel(**inputs) -> np.ndarray:
    from concourse.bass_utils import run_bass_kernel_spmd

    if "nc" not in _CACHE:
        _CACHE["nc"] = _build_nc()
    nc = _CACHE["nc"]

    in_maps = _host_prep(inputs)
    res = run_bass_kernel_spmd(nc, in_maps, core_ids=list(range(NCORES)))
    out = np.zeros((B, L, D), dtype=np.float32)
    for core in range(NCORES):
        b = core // 4
        out[b] += res.results[core]["outp"]
    return out



# revision 8
# speedup vs baseline: 1.0778x; 1.0778x over previous
"""CompressedSparseAttention Trainium2 kernel v2 (8 NeuronCores).

Sharding: data-parallel over batch (2) x tensor-parallel over head-pairs (4).
Core c handles batch b = c//4 and heads (2g, 2g+1) with g = c%4.
Each core computes its partial output attn_out[:, hslice] @ wo[:, hslice].T
([2048, 512] bf16); the host sums the 4 partials per batch in f32.

v2 design notes (vs baseline):
  - all matmul operands bf16 (1 cyc/col on PE)
  - rope swap via host-permuted extra weight streams (wqrot/wkrot), rope
    combine on DVE (no gpsimd partition copies)
  - compressed x_c pooled on host (tiny linear prep, like the x transpose)
  - causal masks via precomputed 0/1 bf16 mask tiles * DVE multiply
    (replaces gpsimd affine_select)
  - AV in [q, dims] layout: av[q, 0:65|65:130] accumulates
    exp(scores)^T @ [v | ones]; col 64/129 = softmax denominators;
    sink contribution via a [1-contraction] outer-product matmul
  - normalization via DVE reciprocal + per-partition scale, then PE
    transpose -> single 128-contraction wo matmul per q-chunk
  - software-pipelined emission: scores/exp of block qb+1 are emitted
    before the AV of block qb (engine streams are in-order)
  - Act engine runs exp ONLY (one activation table, no reloads)
"""

import math

import numpy as np

import concourse.bass as bass
import concourse.mybir as mybir
import concourse.tile as tile
from concourse import bacc
from concourse.bass import ds
from concourse.masks import make_identity

B = 2
L = 2048
D = 512
H = 8
HD = 64
RATIO = 8
STRIDE = 4
WINDOW = 128
THETA = 10000.0
LC = (L - RATIO) // STRIDE + 1  # 511
NCORES = 8
NB = L // 512  # 4 q-blocks of 512
NCH = L // 128  # 16 q-chunks of 128
KD = D // 128  # 4 contraction chunks

F32 = mybir.dt.float32
F32R = mybir.dt.float32r
BF16 = mybir.dt.bfloat16
AF = mybir.ActivationFunctionType
ALU = mybir.AluOpType

_CACHE = {}

# weight column offsets inside wst tiles [128, 896]
WQ, WQR, WK, WKR, WV, WKC, WVC = 0, 128, 256, 384, 512, 640, 768
# const column offsets inside mega tile [128, 6144]
COS0, SIN0, MWIN, MD0, MD1, WOT = 0, 2048, 4096, 4608, 5120, 5632


def _build_nc():
    nc = bacc.Bacc(
        "TRN2",
        target_bir_lowering=False,
        debug=False,
        num_devices=NCORES,
        name="csa2",
    )

    xT_d = nc.dram_tensor("xT", [D, L], BF16, kind="ExternalInput")
    xcT_d = nc.dram_tensor("xcT", [D, LC], BF16, kind="ExternalInput")
    wst_d = nc.dram_tensor("wst", [D, 896], BF16, kind="ExternalInput")
    mega_d = nc.dram_tensor("mega", [128, 6144], BF16, kind="ExternalInput")
    sinkrow_d = nc.dram_tensor("sinkrow", [1, 130], BF16, kind="ExternalInput")
    outp_d = nc.dram_tensor("outp", [L, D], BF16, kind="ExternalOutput")

    with tile.TileContext(nc) as tc:
        with tc.tile_pool(name="consts", bufs=1) as cp, \
             tc.tile_pool(name="work", bufs=1) as wp, \
             tc.tile_pool(name="ps", bufs=5, space="PSUM") as pp, \
             tc.tile_pool(name="pss", bufs=3, space="PSUM") as pps:

            # ---------------- DMAs (in arrival-priority order) ----------
            # one strided DMA per logical tensor: dest packs the 4 dmodel
            # chunks side-by-side along the free dim
            wstt = cp.tile([128, 3584], BF16, tag="wstt")
            nc.sync.dma_start(
                out=wstt,
                in_=wst_d.rearrange("(c p) f -> p c f", c=KD),
            )
            wst = [wstt[:, ds(896 * c, 896)] for c in range(KD)]

            mega = cp.tile([128, 6144], BF16, tag="mega")
            sinkrow = cp.tile([1, 130], BF16, tag="sinkrow")
            xq = []
            xrr = xT_d.rearrange("(c p) (b f) -> p b c f", c=KD, b=NB)
            for qb in range(NB):
                xt = cp.tile([128, 2048], BF16, tag=f"xq{qb}", name=f"xq{qb}")
                nc.sync.dma_start(out=xt, in_=xrr[:, qb])
                xq.append(xt)
                if qb == 0:
                    xct = cp.tile([128, 4 * LC], BF16, tag="xct")
                    nc.sync.dma_start(
                        out=xct,
                        in_=xcT_d.rearrange("(c p) f -> p c f", c=KD),
                    )
                    nc.sync.dma_start(out=mega[:, 0:4096],
                                      in_=mega_d[:, 0:4096])
                elif qb == 1:
                    nc.sync.dma_start(out=mega[:, 4096:6144],
                                      in_=mega_d[:, 4096:6144])
                    nc.sync.dma_start(out=sinkrow, in_=sinkrow_d[:, :])
            xB = [[xq[qb][:, ds(512 * c, 512)] for qb in range(NB)]
                  for c in range(KD)]
            x_cT = [xct[:, ds(LC * c, LC)] for c in range(KD)]

            onesrow = cp.tile([1, 128], BF16, tag="onesrow")
            nc.gpsimd.memset(onesrow, 1.0)
            ident_bf = cp.tile([128, 128], BF16, tag="ident_bf")
            make_identity(nc, ident_bf)

            cosT = mega[:, COS0:COS0 + L]
            sinST = mega[:, SIN0:SIN0 + L]
            mwin = mega[:, MWIN:MWIN + 512]
            md0 = mega[:, MD0:MD0 + 512]
            md1 = mega[:, MD1:MD1 + 512]
            woT = mega[:, WOT:WOT + 512]

            # ---------------- emission helpers ----------------
            qT = cp.tile([128, L], BF16, tag="qT")
            kT = cp.tile([128, L], BF16, tag="kT")

            def emit_proj_qk(qb):
                for woff, wroff, outT in ((WQ, WQR, qT), (WK, WKR, kT)):
                    ps = pp.tile([128, 512], F32, tag="bank", name="proj_ps")
                    for c in range(KD):
                        nc.tensor.matmul(
                            ps,
                            wst[c][:, ds(woff, 128)],
                            xB[c][qb],
                            start=(c == 0),
                            stop=(c == KD - 1),
                        )
                    psr = pp.tile([128, 512], F32, tag="bank", name="projr_ps")
                    for c in range(KD):
                        nc.tensor.matmul(
                            psr,
                            wst[c][:, ds(wroff, 128)],
                            xB[c][qb],
                            start=(c == 0),
                            stop=(c == KD - 1),
                        )
                    m1 = wp.tile([128, 512], BF16, tag="m1", bufs=2, name="m1")
                    nc.vector.tensor_mul(m1, ps, cosT[:, ds(512 * qb, 512)])
                    m2 = wp.tile([128, 512], BF16, tag="m2", bufs=2, name="m2")
                    nc.vector.tensor_mul(m2, psr, sinST[:, ds(512 * qb, 512)])
                    nc.vector.tensor_add(outT[:, ds(512 * qb, 512)], m1, m2)

            v_aug = [None] * NCH

            def emit_v(qb):
                # vT [vdim, pos] for this 512-block, then 4 transposes
                vt_ps = pp.tile([128, 512], F32, tag="bank", name="vt_ps")
                for c in range(KD):
                    nc.tensor.matmul(
                        vt_ps,
                        wst[c][:, ds(WV, 128)],
                        xB[c][qb],
                        start=(c == 0),
                        stop=(c == KD - 1),
                    )
                vt_sb = wp.tile([128, 512], BF16, tag="vt_sb", bufs=2,
                                name="vt_sb")
                nc.scalar.copy(out=vt_sb, in_=vt_ps)
                for sub in range(4):
                    ch = 4 * qb + sub
                    va = cp.tile([128, 130], BF16, tag=f"v_aug{ch}",
                                 name=f"v_aug{ch}")
                    vag = va.rearrange("p (g c) -> p g c", c=65)
                    nc.vector.memset(vag[:, :, 64], 1.0)
                    tp = pps.tile([128, 128], BF16, tag="small", name="v_tp")
                    nc.tensor.transpose(tp, vt_sb[:, ds(128 * sub, 128)],
                                        ident_bf)
                    if sub % 2 == 0:
                        nc.vector.tensor_copy(out=va[:, 0:64], in_=tp[:, 0:64])
                        nc.vector.tensor_copy(out=va[:, 65:129],
                                              in_=tp[:, 64:128])
                    else:
                        nc.scalar.copy(out=va[:, 0:64], in_=tp[:, 0:64])
                        nc.scalar.copy(out=va[:, 65:129], in_=tp[:, 64:128])
                    v_aug[ch] = va

            k_cT = cp.tile([128, LC], BF16, tag="k_cT")
            vc_aug = [None] * 4

            def emit_kcvc():
                kc_ps = pp.tile([128, LC], F32, tag="bank", name="kc_ps")
                for d in range(KD):
                    nc.tensor.matmul(
                        kc_ps,
                        wst[d][:, ds(WKC, 128)],
                        x_cT[d],
                        start=(d == 0),
                        stop=(d == KD - 1),
                    )
                nc.vector.tensor_copy(out=k_cT, in_=kc_ps)
                vc_ps = pp.tile([128, LC], F32, tag="bank", name="vc_ps")
                for d in range(KD):
                    nc.tensor.matmul(
                        vc_ps,
                        wst[d][:, ds(WVC, 128)],
                        x_cT[d],
                        start=(d == 0),
                        stop=(d == KD - 1),
                    )
                v_cT = wp.tile([128, LC], BF16, tag="v_cT", bufs=1,
                               name="v_cT")
                nc.vector.tensor_copy(out=v_cT, in_=vc_ps)
                for ch in range(4):
                    wlen = min(128, LC - 128 * ch)  # 128,128,128,127
                    va = cp.tile([128, 130], BF16, tag=f"vc_aug{ch}",
                                 name=f"vc_aug{ch}")
                    vag = va.rearrange("p (g c) -> p g c", c=65)
                    nc.vector.memset(vag[:, :, 64], 1.0)
                    tp = pps.tile([128, 128], BF16, tag="small", name="vc_tp")
                    nc.tensor.transpose(tp[0:wlen, :],
                                        v_cT[:, ds(128 * ch, wlen)], ident_bf)
                    nc.scalar.copy(out=va[0:wlen, 0:64], in_=tp[0:wlen, 0:64])
                    nc.scalar.copy(out=va[0:wlen, 65:129],
                                   in_=tp[0:wlen, 64:128])
                    vc_aug[ch] = va

            def emit_scores(qb):
                exc = [[None] * (qb + 1) for _ in range(2)]
                exw = [[None, None] for _ in range(2)]
                brs = [None, None]
                for h in range(2):
                    hs = 64 * h
                    qs = qT[ds(hs, 64), ds(512 * qb, 512)]
                    for wc in range(qb + 1):
                        wlen = min(128, LC - 128 * wc)
                        sc = pp.tile([128, 512], F32, tag="bank", name="sc_ps")
                        nc.tensor.matmul(
                            sc[0:wlen, :],
                            k_cT[ds(hs, 64), ds(128 * wc, wlen)],
                            qs,
                            start=True,
                            stop=True,
                        )
                        ex = wp.tile([128, 512], BF16, tag="exc", bufs=20,
                                     name="exc")
                        if wc >= qb - 1:
                            exr = wp.tile([128, 512], BF16, tag="exr", bufs=3,
                                          name="exr")
                            nc.scalar.activation(
                                out=exr[0:wlen, :], in_=sc[0:wlen, :],
                                func=AF.Exp, scale=0.125,
                            )
                            msk = md0 if wc == qb else md1
                            nc.vector.tensor_mul(
                                ex[0:wlen, :], exr[0:wlen, :], msk[0:wlen, :]
                            )
                        else:
                            nc.scalar.activation(
                                out=ex[0:wlen, :], in_=sc[0:wlen, :],
                                func=AF.Exp, scale=0.125,
                            )
                        exc[h][wc] = ex

                    for p in range(2):
                        kc0 = 4 * qb + 2 * p
                        ncols = 512 if kc0 < 14 else 384
                        wps = pp.tile([128, 512], F32, tag="bank", name="win_ps")
                        for j in range(2):
                            kc = kc0 + j
                            qcols = min(256, L - 128 * kc)
                            nc.tensor.matmul(
                                wps[:, ds(256 * j, qcols)],
                                kT[ds(hs, 64), ds(128 * kc, 128)],
                                qT[ds(hs, 64), ds(128 * kc, qcols)],
                                start=True,
                                stop=True,
                                skip_group_check=True,
                            )
                        ewr = wp.tile([128, 512], BF16, tag="ewr", bufs=3,
                                      name="ewr")
                        nc.scalar.activation(
                            out=ewr[:, 0:ncols], in_=wps[:, 0:ncols],
                            func=AF.Exp, scale=0.125,
                        )
                        ew = wp.tile([128, 512], BF16, tag="exw", bufs=16,
                                     name="exw")
                        nc.vector.tensor_mul(
                            ew[:, 0:ncols], ewr[:, 0:ncols], mwin[:, 0:ncols]
                        )
                        exw[h][p] = ew

                    if qb >= 1:
                        # bridge: keys chunk 4qb-1 vs q-chunk 4qb (replaces
                        # the cross-qb prev_exw handoff)
                        kc = 4 * qb - 1
                        bps = pps.tile([128, 128], F32, tag="small", name="bps")
                        nc.tensor.matmul(
                            bps,
                            kT[ds(hs, 64), ds(128 * kc, 128)],
                            qT[ds(hs, 64), ds(128 * (kc + 1), 128)],
                            start=True,
                            stop=True,
                        )
                        bre = wp.tile([128, 128], BF16, tag="bre", bufs=2,
                                      name="bre")
                        nc.scalar.activation(
                            out=bre, in_=bps, func=AF.Exp, scale=0.125,
                        )
                        br = wp.tile([128, 128], BF16, tag="br", bufs=4,
                                     name="br")
                        nc.vector.tensor_mul(br, bre, mwin[:, 128:256])
                        brs[h] = br
                return exc, exw, brs

            def emit_av_mms(qb, sub, exc, exw, brs):
                c = 4 * qb + sub
                av = pps.tile([128, 130], F32, tag="small", name="av")
                nc.tensor.matmul(
                    av, onesrow, sinkrow, start=True, stop=False,
                    skip_group_check=True,
                )
                for h in range(2):
                    last_h = (h == 1)
                    cols = ds(65 * h, 65)
                    pcur, jcur = sub // 2, sub % 2
                    nc.tensor.matmul(
                        av[:, cols],
                        exw[h][pcur][:, ds(256 * jcur, 128)],
                        v_aug[c][:, cols],
                        start=False, stop=False,
                        skip_group_check=True,
                    )
                    if c > 0:
                        if sub == 0:
                            epv = brs[h][:, 0:128]
                        else:
                            ew_, jpv = exw[h][(sub - 1) // 2], (sub - 1) % 2
                            epv = ew_[:, ds(256 * jpv + 128, 128)]
                        nc.tensor.matmul(
                            av[:, cols],
                            epv,
                            v_aug[c - 1][:, cols],
                            start=False, stop=False,
                            skip_group_check=True,
                        )
                    for wc in range(qb + 1):
                        wlen = min(128, LC - 128 * wc)
                        nc.tensor.matmul(
                            av[:, cols],
                            exc[h][wc][0:wlen, ds(128 * sub, 128)],
                            vc_aug[wc][0:wlen, cols],
                            start=False,
                            stop=(last_h and wc == qb),
                            skip_group_check=True,
                        )
                return av

            outsb = [None] * NB

            def emit_av_chain(av, c):
                avg = av.rearrange("p (g c) -> p g c", c=65)
                rec2 = wp.tile([128, 2], F32, tag="rec2", bufs=3, name="rec2")
                nc.vector.reciprocal(out=rec2, in_=avg[:, :, 64])
                avn = wp.tile([128, 128], BF16, tag="avn", bufs=3, name="avn")
                for h in range(2):
                    nc.vector.tensor_scalar(
                        out=avn[:, ds(64 * h, 64)],
                        in0=av[:, ds(65 * h, 64)],
                        scalar1=rec2[:, ds(h, 1)],
                        scalar2=None,
                        op0=ALU.mult,
                    )
                trp = pps.tile([128, 128], BF16, tag="small", name="trp")
                nc.tensor.transpose(trp, avn, ident_bf)
                avT = wp.tile([128, 128], BF16, tag="avT", bufs=2, name="avT")
                if c % 2 == 0:
                    nc.vector.tensor_copy(out=avT, in_=trp)
                else:
                    nc.scalar.copy(out=avT, in_=trp)
                po = pp.tile([128, 512], F32, tag="bank", name="po")
                nc.tensor.matmul(po, avT, woT, start=True, stop=True)
                qb, sub = c // 4, c % 4
                if sub == 0:
                    outsb[qb] = wp.tile([128, 2048], BF16, tag="osb", bufs=2,
                                        name="osb")
                dst = outsb[qb][:, ds(512 * sub, 512)]
                if sub % 2 == 0:
                    nc.scalar.copy(out=dst, in_=po)
                else:
                    nc.vector.tensor_copy(out=dst, in_=po)
                if sub == 3:
                    nc.sync.dma_start(
                        out=outp_d.rearrange("(b s p) f -> p b s f",
                                             b=NB, s=4)[:, qb],
                        in_=outsb[qb],
                    )

            def emit_av_block(qb, exc, exw, brs):
                avs = [None] * 4
                for sub in range(4):
                    avs[sub] = emit_av_mms(qb, sub, exc, exw, brs)
                    if sub >= 1:
                        emit_av_chain(avs[sub - 1], 4 * qb + sub - 1)
                emit_av_chain(avs[3], 4 * qb + 3)

            # ---------------- main emission (qb-major) -------------------
            tiles = {}
            for qb in range(NB):
                emit_proj_qk(qb)
                if qb == 0:
                    emit_kcvc()
                tiles[qb] = emit_scores(qb)
                emit_v(qb)
                if qb >= 1:
                    exc, exw, brs = tiles.pop(qb - 1)
                    emit_av_block(qb - 1, exc, exw, brs)
            exc, exw, brs = tiles.pop(NB - 1)
            emit_av_block(NB - 1, exc, exw, brs)

    nc.compile()
    return nc


def _host_prep(inputs):
    """Build the 8 per-core input maps from full inputs."""
    import ml_dtypes

    bf = ml_dtypes.bfloat16
    x = np.asarray(inputs["x"], dtype=np.float32)
    wq = np.asarray(inputs["wq"], dtype=np.float32)
    wk = np.asarray(inputs["wk"], dtype=np.float32)
    wv = np.asarray(inputs["wv"], dtype=np.float32)
    wo = np.asarray(inputs["wo"], dtype=np.float32)
    wk_c = np.asarray(inputs["wk_c"], dtype=np.float32)
    wv_c = np.asarray(inputs["wv_c"], dtype=np.float32)
    gate_logits = np.asarray(inputs["gate_logits"], dtype=np.float32)
    sink_logit = np.asarray(inputs["sink_logit"], dtype=np.float32)

    # rope tables
    half = HD // 2
    inv_freq = 1.0 / (THETA ** (np.arange(half, dtype=np.float32) / half))
    t = np.arange(L, dtype=np.float32)
    f = t[:, None] * inv_freq[None, :]  # [L, 32]
    cos32 = np.cos(f).T.astype(np.float32)  # [32, L]
    sin32 = np.sin(f).T.astype(np.float32)
    cosT = np.tile(cos32, (4, 1))  # rows: i%32
    sinST = np.concatenate([-sin32, sin32, -sin32, sin32], axis=0)

    # masks
    qr = np.arange(512)
    kr = np.arange(128)
    low = (qr[None, 0:128] >= kr[:, None]).astype(np.float32)   # cur chunk
    upp = (kr[:, None] > qr[None, 0:128]).astype(np.float32)    # prev chunk
    mwin = np.concatenate([low, upp, low, upp], axis=1)  # [128, 512]
    md0 = (qr[None, :] >= 4 * kr[:, None] + 7).astype(np.float32)
    md1 = (qr[None, :] >= 4 * kr[:, None] - 505).astype(np.float32)

    mega = np.concatenate(
        [cosT, sinST, mwin, md0, md1, np.zeros((128, 512), np.float32)],
        axis=1,
    )  # [128, 6144]; wo slice filled per core

    # compressed pooling on host: x_c[b, w] = sum_r g_r x[b, 4w+r]
    g = np.exp(gate_logits - gate_logits.max())
    g = (g / g.sum()).astype(np.float32)
    idx = np.arange(RATIO)[None, :] + np.arange(LC)[:, None] * STRIDE  # [LC, 8]
    x_c = np.einsum("bwrd,r->bwd", x[:, idx, :], g)  # [B, LC, D]

    # rope permutation of projection output dims (within the 128-dim slice):
    # row j of the rotated weights = row swap(j): 0..31 <-> 32..63 per head
    perm = np.arange(128).reshape(2, 2, 32)[:, ::-1, :].reshape(128)

    in_maps = []
    for core in range(NCORES):
        b, grp = divmod(core, 4)
        sl = slice(128 * grp, 128 * (grp + 1))
        m = mega.copy()
        m[:, WOT:WOT + 512] = wo[:, sl].T
        wq_s = wq[sl]
        wk_s = wk[sl]
        wst = np.concatenate(
            [wq_s.T, wq_s[perm].T, wk_s.T, wk_s[perm].T,
             wv[sl].T, wk_c[sl].T, wv_c[sl].T], axis=1
        )  # [512, 896]
        sinkrow = np.zeros((1, 130), np.float32)
        sinkrow[0, 64] = np.exp(sink_logit[2 * grp, 0])
        sinkrow[0, 129] = np.exp(sink_logit[2 * grp + 1, 0])
        in_maps.append(
            {
                "xT": np.ascontiguousarray(x[b].T).astype(bf),
                "xcT": np.ascontiguousarray(x_c[b].T).astype(bf),
                "wst": np.ascontiguousarray(wst).astype(bf),
                "mega": m.astype(bf),
                "sinkrow": sinkrow.astype(bf),
            }
        )
    return in_maps


def kernel(**inputs) -> np.ndarray:
    from concourse.bass_utils import run_bass_kernel_spmd

    if "nc" not in _CACHE:
        _CACHE["nc"] = _build_nc()
    nc = _CACHE["nc"]

    in_maps = _host_prep(inputs)
    res = run_bass_kernel_spmd(nc, in_maps, core_ids=list(range(NCORES)))
    out = np.zeros((B, L, D), dtype=np.float32)
    for core in range(NCORES):
        b = core // 4
        out[b] += np.asarray(res.results[core]["outp"], dtype=np.float32)
    return out
